# revision 7
# baseline (speedup 1.0000x reference)
"""Trainium2 Bass kernel: ODE-RNN encoder (z0 encoder), data-parallel over batch.

Strategy (v2 — selective fp8 DoubleRow + fully unrolled time loop)
------------------------------------------------------------------
- 8 NeuronCores, batch (n_traj=2048) sharded 256/core; weights replicated.
- Feature-major on chip: tiles are [feature_chunk(128), batch(256)]; fused
  chunk-pair tiles [128, 2, 256] double as the fp8 DoubleRow moving operand.
- Per-GEMM precision flags (U8/R8/N18/N28): flagged GEMMs run fp8(e4m3)
  with perf_mode=DoubleRow (2 k-chunks per MM, ~1.5x PE throughput);
  weights pre-scaled x16 host-side (avoids e4m3 subnormals), the 1/16
  folded into activation `scale` or vector-op scalar multipliers. The ODE
  MLP second matmul is always fp8-DR: its output is scaled by dt~0.005, so
  quantization there is measurably free. x and the x-part weights stay f16
  (the 5th k-chunk is a single MM either way - no speed loss).
- Master carries y,s stay fp16; e4m3 shadow copies (s8, yo8) feed fp8
  matmuls, so fp8 error does not accumulate through the recurrence.
- Time loop FULLY UNROLLED (200 static steps): no hardware-loop barriers,
  no per-iteration act-table reloads, dt baked in as immediates.
- All xs preloaded to SBUF in 8 chunked DMAs up front: zero DMA in-loop.
- Build-time specialization on actual inputs: all-zero biases (per spec
  fill=zeros) -> bias-free fused [128,512]-wide activations; all-ones
  observation mask (P[all 64 mask bits 0] ~ 0.2^64) -> mask dropped.
  Nonzero bias / nonuniform mask fall back to general (slower) paths.
- Gating algebra: Wu2 negated host-side so sigmoid gives v = 1-u directly:
  ny = y_ode + v*(new_state - y_ode), nstd = s + v*(|ns'| - s); abs via
  AluOp abs_max(x, 0); final clip via abs_max(z, 1e-20).
"""

import os
import sys

import numpy as np
import ml_dtypes

if "/opt/trn_rl_repo" not in sys.path:
    sys.path.insert(0, "/opt/trn_rl_repo")

import concourse.bacc as bacc
import concourse.bass as bass
import concourse.mybir as mybir
from concourse import tile
from concourse.alu_op_type import AluOpType
from concourse.bass_utils import run_bass_kernel_spmd

F32 = mybir.dt.float32
F16 = mybir.dt.float16
F8 = mybir.dt.float8e4
AF = mybir.ActivationFunctionType
DR = mybir.MatmulPerfMode.DoubleRow
NPF8 = ml_dtypes.float8_e4m3

B, NT, IN = 2048, 200, 128
LAT, NU = 256, 512
DHALF = IN // 2
NCORES = 8
BC = B // NCORES  # 256 trajectories per core
WS = 16.0  # fp8 weight pre-scale (keeps weights out of e4m3 subnormals)
NQ = 8     # xs DMA chunks

# Per-GEMM fp8 flags (u-gate, r-gate, n first layer, n second layer).
U8 = True
R8 = True
N18 = False
N28 = False

_last_results = None


class _Bacc(bacc.Bacc):
    def insert_act_table_loads(self):
        import concourse.mybir as mb
        from concourse.bacc import _bass_rust
        from concourse.hw_specs import get_activation_tables

        has_activation = any(
            isinstance(i, mb.InstActivation)
            for b in self.main_func.blocks
            for i in b.instructions
        )
        if not has_activation:
            return
        tables = []
        for name, funcs in get_activation_tables(self.m.arch).items():
            # keep positions (act_func_set_id is positional) but only let
            # sigmoid_and_others match, so one load covers the whole program
            tables.append((name, funcs if name == "sigmoid_and_others" else set()))
        _bass_rust.insert_act_table_loads(self, tables)


def build_program(nt, dts, zero_bias=True, mask_ones=True):
    """Build the single-core SPMD Bass program. dts: np.ndarray [nt] of per-step
    Euler dt (baked as immediates). Returns (nc, input_names)."""
    nc = _Bacc(
        trn_type="TRN2",
        target_bir_lowering=False,
        debug=False,
        enable_asserts=False,
    )
    ADD, SUB, MUL = AluOpType.add, AluOpType.subtract, AluOpType.mult
    AMAX = AluOpType.abs_max
    BYP = AluOpType.bypass

    d = {}

    def inp(name, shape, dt):
        d[name] = nc.dram_tensor(name, shape, dt, kind="ExternalInput").ap()
        return d[name]

    QS = (nt + NQ - 1) // NQ
    xs_d = inp("xs", [128, NQ * QS, BC], F16)

    def w1shapes(f8):
        # (x-part [128,4,128] f16) + pair parts: DR [128,4,2,128] f8
        # or f16 4-chunk [128,4,2,128] (kf within pair; same shape)
        return ([128, 4, 128], [128, 4, 2, 128])

    wo1_d = inp("wo1", [128, 2, 2, 128], F16)
    wo2_d = inp("wo2", [128, 2, 2, 128], F8)
    wu1x_d = inp("wu1x", [128, 4, 128], F16)
    wu1s_d = inp("wu1s", [128, 4, 2, 128], F8 if U8 else F16)
    wu1y_d = inp("wu1y", [128, 4, 2, 128], F8 if U8 else F16)
    wr1x_d = inp("wr1x", [128, 4, 128], F16)
    wr1s_d = inp("wr1s", [128, 4, 2, 128], F8 if R8 else F16)
    wr1y_d = inp("wr1y", [128, 4, 2, 128], F8 if R8 else F16)
    wn1x_d = inp("wn1x", [128, 4, 128], F16)
    wn1yr_d = inp("wn1yr", [128, 4, 2, 128], F8 if N18 else F16)
    wn1sr_d = inp("wn1sr", [128, 4, 2, 128], F8 if N18 else F16)
    wu2_d = inp("wu2", [128, 2, 2, 2, 128], F8 if U8 else F16)
    wr2_d = inp("wr2", [128, 2, 2, 2, 128], F8 if R8 else F16)
    wn2_d = inp("wn2", [128, 4, 2, 2, 128], F8 if N28 else F16)
    wt1_d = inp("wt1", [128, 4, 100], F16)
    wt2_d = inp("wt2", [100, 4, 128], F16)

    if not zero_bias:
        bo1_d = inp("bo1c", [128, 2], F32)
        dtbo2_d = inp("dtbo2", [128, 2 * nt], F32)
        bu1_d = inp("bu1c", [128, 4], F32)
        bu2n_d = inp("bu2nc", [128, 2], F32)
        br1_d = inp("br1c", [128, 4], F32)
        br2_d = inp("br2c", [128, 2], F32)
        bn1_d = inp("bn1c", [128, 4], F32)
        bn2_d = inp("bn2c", [128, 4], F32)
        bt1_d = inp("bt1c", [100, 1], F32)
        bt2_d = inp("bt2c", [128, 4], F32)
    if not mask_ones:
        maskw_d = inp("maskw", [128, 128], F16)

    om_d = nc.dram_tensor("out_mean", [LAT, BC], F32, kind="ExternalOutput").ap()
    os_d = nc.dram_tensor("out_std", [LAT, BC], F32, kind="ExternalOutput").ap()

    with tile.TileContext(nc) as tc:
        with (
            tc.tile_pool(name="wpool", bufs=1) as wpool,
            tc.tile_pool(name="xpool", bufs=1) as xpool,
            tc.tile_pool(name="cpool", bufs=1) as cpool,
            tc.tile_pool(name="spool", bufs=3) as spool,
            tc.tile_pool(name="pspool", bufs=8, space=bass.MemorySpace.PSUM) as pspool,
        ):
            def load(name, dram, shape, dt):
                t = wpool.tile(shape, dt, name=name, tag=name)
                nc.sync.dma_start(t[:], dram[:])
                return t

            wo1 = load("wo1", wo1_d, [128, 2, 2, 128], F16)
            wo2 = load("wo2", wo2_d, [128, 2, 2, 128], F8)
            wu1x = load("wu1x", wu1x_d, [128, 4, 128], F16)
            wu1s = load("wu1s", wu1s_d, [128, 4, 2, 128], F8 if U8 else F16)
            wu1y = load("wu1y", wu1y_d, [128, 4, 2, 128], F8 if U8 else F16)
            wr1x = load("wr1x", wr1x_d, [128, 4, 128], F16)
            wr1s = load("wr1s", wr1s_d, [128, 4, 2, 128], F8 if R8 else F16)
            wr1y = load("wr1y", wr1y_d, [128, 4, 2, 128], F8 if R8 else F16)
            wn1x = load("wn1x", wn1x_d, [128, 4, 128], F16)
            wn1yr = load("wn1yr", wn1yr_d, [128, 4, 2, 128], F8 if N18 else F16)
            wn1sr = load("wn1sr", wn1sr_d, [128, 4, 2, 128], F8 if N18 else F16)
            wu2 = load("wu2", wu2_d, [128, 2, 2, 2, 128], F8 if U8 else F16)
            wr2 = load("wr2", wr2_d, [128, 2, 2, 2, 128], F8 if R8 else F16)
            wn2 = load("wn2", wn2_d, [128, 4, 2, 2, 128], F8 if N28 else F16)
            wt1 = load("wt1", wt1_d, [128, 4, 100], F16)
            wt2 = load("wt2", wt2_d, [100, 4, 128], F16)
            if not zero_bias:
                bo1 = load("bo1", bo1_d, [128, 2], F32)
                dtbo2 = load("dtbo2", dtbo2_d, [128, 2 * nt], F32)
                bu1 = load("bu1", bu1_d, [128, 4], F32)
                bu2n = load("bu2n", bu2n_d, [128, 2], F32)
                br1 = load("br1", br1_d, [128, 4], F32)
                br2 = load("br2", br2_d, [128, 2], F32)
                bn1 = load("bn1", bn1_d, [128, 4], F32)
                bn2 = load("bn2", bn2_d, [128, 4], F32)
                bt1 = load("bt1", bt1_d, [100, 1], F32)
                bt2 = load("bt2", bt2_d, [128, 4], F32)
            if not mask_ones:
                maskw = load("maskw", maskw_d, [128, 128], F16)

            xq = []
            for k in range(NQ):
                t = xpool.tile([128, QS, BC], F16, name=f"xq{k}", tag=f"xq{k}")
                nc.sync.dma_start(t[:], xs_d[:, k * QS : (k + 1) * QS])
                xq.append(t)

            # Carries: fp16 masters (+ e4m3 shadow of s if needed).
            y16 = cpool.tile([128, 2, BC], F16, name="y16", tag="y16")
            s16 = cpool.tile([128, 2, BC], F16, name="s16", tag="s16")
            inits = [y16, s16]
            need_shadow = U8 or R8
            if need_shadow:
                s8 = cpool.tile([128, 2, BC], F8, name="s8", tag="s8")
                inits.append(s8)
            for t in inits:
                nc.vector.memset(t[:], 0.0)

            TT = nc.vector.tensor_tensor
            TS = nc.vector.tensor_scalar
            STT = nc.vector.scalar_tensor_tensor
            MM = nc.tensor.matmul
            ACT = nc.scalar.activation

            def ps():
                return pspool.tile([128, 2, BC], F32, name="ps", tag="ps")

            def act_pair(out_t, ps_t, func, scale, bcols, bidx):
                if zero_bias:
                    ACT(out_t[:], ps_t[:], func, scale=scale)
                else:
                    for c in range(2):
                        ACT(out_t[:, c], ps_t[:, c], func,
                            bias=bcols[:, bidx + c : bidx + c + 1], scale=scale)

            def layer1(wx, wp1, mov1, wp2, mov2, x16, fp8):
                """[x | part1 | part2] @ W1 -> two psum banks (4 out chunks).
                fp8: pair parts as DR; else two f16 MMs per pair."""
                banks = []
                for bk in range(2):
                    p = ps()
                    for c in range(2):
                        mf = bk * 2 + c
                        MM(p[:, c], wx[:, mf], x16, start=True, stop=False)
                        if fp8:
                            MM(p[:, c], wp1[:, mf], mov1[:], start=False,
                               stop=False, perf_mode=DR)
                            MM(p[:, c], wp2[:, mf], mov2[:], start=False,
                               stop=True, perf_mode=DR)
                        else:
                            for kj in range(2):
                                MM(p[:, c], wp1[:, mf, kj], mov1[:, kj],
                                   start=False, stop=False)
                            for kj in range(2):
                                MM(p[:, c], wp2[:, mf, kj], mov2[:, kj],
                                   start=False, stop=(kj == 1))
                    banks.append(p)
                return banks

            def layer2(w, h, n_mf, fp8):
                banks = []
                for bk in range(n_mf // 2):
                    p = ps()
                    for c in range(2):
                        mf = bk * 2 + c
                        if fp8:
                            for kp in range(2):
                                MM(p[:, c], w[:, mf, kp], h[kp][:],
                                   start=(kp == 0), stop=(kp == 1), perf_mode=DR)
                        else:
                            for kp in range(2):
                                for kj in range(2):
                                    MM(p[:, c], w[:, mf, kp, kj], h[kp][:, kj],
                                       start=(kp == 0 and kj == 0),
                                       stop=(kp == 1 and kj == 1))
                    banks.append(p)
                return banks

            def step(t):
                dt = float(dts[t])
                x16 = xq[t // QS][:, t % QS]

                # --- ODE: y_ode = y + dt * (tanh(y@Wo1) @ Wo2)
                pso1 = ps()
                for mf in range(2):
                    for kf in range(2):
                        MM(pso1[:, mf], wo1[:, mf, kf], y16[:, kf],
                           start=(kf == 0), stop=(kf == 1))
                ho8 = spool.tile([128, 2, BC], F8, name="ho8", tag="ho8")
                act_pair(ho8, pso1, AF.Tanh, 1.0, bo1 if not zero_bias else None, 0)
                pso2 = ps()
                for mf in range(2):
                    MM(pso2[:, mf], wo2[:, mf], ho8[:], start=True, stop=True,
                       perf_mode=DR)
                yo16 = spool.tile([128, 2, BC], F16, name="yo16", tag="yo16")
                STT(yo16[:], pso2[:], dt / WS, y16[:], MUL, ADD)
                if not zero_bias:
                    for c in range(2):
                        TS(yo16[:, c], yo16[:, c],
                           dtbo2[:, t + c * nt : t + c * nt + 1], None, ADD, BYP)
                if need_shadow:
                    yo8 = spool.tile([128, 2, BC], F8, name="yo8", tag="yo8")
                    nc.vector.tensor_copy(yo8[:], yo16[:])

                # --- observation mask (fallback only; m == 1 on spec inputs)
                if not mask_ones:
                    pm = ps()
                    MM(pm[:, 0], maskw[:], x16, start=True, stop=True)
                    mb16 = spool.tile([128, BC], F16, name="mb16", tag="mb16")
                    TS(mb16[:], pm[:, 0], 0.0, None, AluOpType.is_gt, BYP)

                # --- u (update) and r (reset) gates
                su = 1.0 / WS if U8 else 1.0
                sr_ = 1.0 / WS if R8 else 1.0
                sn = 1.0 / WS if N18 else 1.0
                sn2 = 1.0 / WS if N28 else 1.0
                psu = layer1(wu1x, wu1s, s8 if U8 else s16, wu1y,
                             yo8 if U8 else yo16, x16, U8)
                psr = layer1(wr1x, wr1s, s8 if R8 else s16, wr1y,
                             yo8 if R8 else yo16, x16, R8)
                hu = []
                for bk in range(2):
                    h = spool.tile([128, 2, BC], F8 if U8 else F16,
                                   name=f"hu{bk}", tag=f"hu{bk}")
                    act_pair(h, psu[bk], AF.Tanh, su,
                             bu1 if not zero_bias else None, bk * 2)
                    hu.append(h)
                hr = []
                for bk in range(2):
                    h = spool.tile([128, 2, BC], F8 if R8 else F16,
                                   name=f"hr{bk}", tag=f"hr{bk}")
                    act_pair(h, psr[bk], AF.Tanh, sr_,
                             br1 if not zero_bias else None, bk * 2)
                    hr.append(h)

                psu2 = layer2(wu2, hu, 2, U8)[0]
                v16 = spool.tile([128, 2, BC], F16, name="v16", tag="v16")
                # Wu2 negated host-side: sigmoid(-z) = 1-u = v
                act_pair(v16, psu2, AF.Sigmoid, su,
                         bu2n if not zero_bias else None, 0)
                psr2 = layer2(wr2, hr, 2, R8)[0]
                r16 = spool.tile([128, 2, BC], F16, name="r16", tag="r16")
                act_pair(r16, psr2, AF.Sigmoid, sr_,
                         br2 if not zero_bias else None, 0)

                if not mask_ones:
                    for c in range(2):
                        TT(v16[:, c], v16[:, c], mb16[:], MUL)

                ydt = F8 if N18 else F16
                yr_ = spool.tile([128, 2, BC], ydt, name="yr", tag="yr")
                TT(yr_[:], yo16[:], r16[:], MUL)
                sr8 = spool.tile([128, 2, BC], ydt, name="sr", tag="sr")
                TT(sr8[:], s16[:], r16[:], MUL)

                # --- candidate state/std
                psn = layer1(wn1x, wn1yr, yr_, wn1sr, sr8, x16, N18)
                hn = []
                for bk in range(2):
                    h = spool.tile([128, 2, BC], F8 if N28 else F16,
                                   name=f"hn{bk}", tag=f"hn{bk}")
                    act_pair(h, psn[bk], AF.Tanh, sn,
                             bn1 if not zero_bias else None, bk * 2)
                    hn.append(h)
                psn2 = layer2(wn2, hn, 4, N28)  # bank0: new_state, bank1: new_std

                # --- gating: ny = yo + v*(ns - yo); nstd = s + v*(|ns'| - s)
                dd = spool.tile([128, 2, BC], F16, name="dd", tag="dd")
                if zero_bias:
                    STT(dd[:], psn2[0][:], sn2, yo16[:], MUL, SUB)
                else:
                    for c in range(2):
                        TS(dd[:, c], psn2[0][:, c], sn2,
                           bn2[:, c : c + 1], MUL, ADD)
                    TT(dd[:], dd[:], yo16[:], SUB)
                t2 = spool.tile([128, 2, BC], F16, name="t2", tag="t2")
                TT(t2[:], v16[:], dd[:], MUL)
                TT(y16[:], yo16[:], t2[:], ADD)

                ab = spool.tile([128, 2, BC], F16, name="ab", tag="ab")
                if zero_bias:
                    ACT(ab[:], psn2[1][:], AF.Abs, scale=sn2)
                else:
                    for c in range(2):
                        ACT(ab[:, c], psn2[1][:, c], AF.Abs,
                            bias=bn2[:, 2 + c : 3 + c], scale=sn2)
                d2 = spool.tile([128, 2, BC], F16, name="d2", tag="d2")
                TT(d2[:], ab[:], s16[:], SUB)
                t3 = spool.tile([128, 2, BC], F16, name="t3", tag="t3")
                TT(t3[:], v16[:], d2[:], MUL)
                TT(s16[:], s16[:], t3[:], ADD)
                if need_shadow:
                    nc.vector.tensor_copy(s8[:], s16[:])

            for t in range(nt):
                step(t)

            # --- head: z = tanh([y,s]@Wt1 + bt1) @ Wt2 + bt2
            pz = ps()
            movs = [y16[:, 0], y16[:, 1], s16[:, 0], s16[:, 1]]
            for kf in range(4):
                MM(pz[:100, 0], wt1[:, kf], movs[kf],
                   start=(kf == 0), stop=(kf == 3))
            h1 = spool.tile([100, BC], F16, name="h1", tag="h1")
            if zero_bias:
                ACT(h1[:], pz[:100, 0], AF.Tanh)
            else:
                ACT(h1[:], pz[:100, 0], AF.Tanh, bias=bt1[:, 0:1])
            for mf in range(4):
                p2 = ps()
                MM(p2[:, 0], wt2[:, mf], h1[:], start=True, stop=True)
                o = spool.tile([128, BC], F32, name=f"zo{mf}", tag=f"zo{mf}")
                if mf < 2:
                    if zero_bias:
                        nc.vector.tensor_copy(o[:], p2[:, 0])
                    else:
                        TS(o[:], p2[:, 0], bt2[:, mf : mf + 1], None, ADD, BYP)
                    nc.sync.dma_start(om_d[mf * 128 : (mf + 1) * 128, :], o[:])
                else:
                    if zero_bias:
                        ACT(o[:], p2[:, 0], AF.Abs)
                    else:
                        ACT(o[:], p2[:, 0], AF.Abs, bias=bt2[:, mf : mf + 1])
                    oc = spool.tile([128, BC], F32, name=f"zc{mf}", tag=f"zc{mf}")
                    nc.vector.tensor_scalar_max(oc[:], o[:], 1e-20)
                    nc.sync.dma_start(os_d[(mf - 2) * 128 : (mf - 1) * 128, :], oc[:])

    nc.compile()
    return nc, list(d.keys())


def _dr_pack(W, scale, fp8):
    """W [K, M] (K%256==0) -> [128, M//128, K//256, 2, 128], DR pair layout.
    (The f16 4D variant uses the same layout: [.., kf within pair, ..].)"""
    K, M = W.shape
    a = np.asarray(W, np.float32).reshape(K // 256, 2, 128, M // 128, 128) * scale
    a = np.ascontiguousarray(a.transpose(2, 3, 0, 1, 4))
    return a.astype(NPF8) if fp8 else a.astype(np.float16)


def _x_pack(W, scale):
    """W [128, M] -> [128, M//128, 128] f16, scaled."""
    M = W.shape[1]
    return np.ascontiguousarray(
        np.asarray(W, np.float32).reshape(128, M // 128, 128) * scale
    ).astype(np.float16)


def _f16_pack(W):
    """W [K, M] (mult of 128) -> [128, M//128, K//128, 128] fp16."""
    K, M = W.shape
    a = np.asarray(W, np.float32).reshape(K // 128, 128, M // 128, 128)
    return np.ascontiguousarray(a.transpose(1, 2, 0, 3)).astype(np.float16)


def _bcols(b, p=128):
    b = np.asarray(b, np.float32)
    n = b.shape[0]
    if n % p != 0:
        return np.ascontiguousarray(b.reshape(n, 1))
    return np.ascontiguousarray(b.reshape(n // p, p).T)


def make_inputs(data, time_steps, Wu1, bu1, Wu2, bu2, Wr1, br1, Wr2, br2,
                Wn1, bn1, Wn2, bn2, Wo1, bo1, Wo2, bo2, Wt1, bt1, Wt2, bt2,
                nt=None, ncores=NCORES, zero_bias=True, mask_ones=True):
    """Host-side shard/layout prep. Returns (list of per-core input dicts, dts)."""
    f = np.float32
    data = np.asarray(data, f)
    time_steps = np.asarray(time_steps, f)
    nt = data.shape[1] if nt is None else nt

    dts = np.concatenate([np.array([-0.01], f),
                          (time_steps[:-1] - time_steps[1:])[::-1]]).astype(f)
    assert dts.shape[0] == nt

    Wu1, Wr1, Wn1 = (np.asarray(w, f) for w in (Wu1, Wr1, Wn1))
    su = WS if U8 else 1.0
    sr = WS if R8 else 1.0
    sn = WS if N18 else 1.0
    sn2 = WS if N28 else 1.0

    def sq(a):
        return np.squeeze(a, axis=2)

    shared = dict(
        wo1=_f16_pack(np.asarray(Wo1, f)),
        wo2=sq(_dr_pack(np.asarray(Wo2, f), WS, True)),
        wu1x=_x_pack(Wu1[2 * LAT:], su),
        wu1s=sq(_dr_pack(Wu1[LAT : 2 * LAT], su, U8)),
        wu1y=sq(_dr_pack(Wu1[:LAT], su, U8)),
        wr1x=_x_pack(Wr1[2 * LAT:], sr),
        wr1s=sq(_dr_pack(Wr1[LAT : 2 * LAT], sr, R8)),
        wr1y=sq(_dr_pack(Wr1[:LAT], sr, R8)),
        wn1x=_x_pack(Wn1[2 * LAT:], sn),
        wn1yr=sq(_dr_pack(Wn1[:LAT], sn, N18)),
        wn1sr=sq(_dr_pack(Wn1[LAT : 2 * LAT], sn, N18)),
        wu2=_dr_pack(-np.asarray(Wu2, f), su, U8),  # negated: sigmoid -> 1-u
        wr2=_dr_pack(np.asarray(Wr2, f), sr, R8),
        wn2=_dr_pack(np.asarray(Wn2, f), sn2, N28),
        wt1=np.ascontiguousarray(
            np.asarray(Wt1, f).reshape(4, 128, 100).transpose(1, 0, 2)
        ).astype(np.float16),
        wt2=np.ascontiguousarray(
            np.asarray(Wt2, f).reshape(100, 4, 128)
        ).astype(np.float16),
    )
    if not zero_bias:
        bo2c2 = np.asarray(bo2, f).reshape(2, 128)
        dtbo2 = np.empty((128, 2 * nt), f)
        for c in range(2):
            dtbo2[:, c * nt : (c + 1) * nt] = bo2c2[c][:, None] * dts[None, :]
        shared.update(
            bo1c=_bcols(bo1), dtbo2=dtbo2,
            bu1c=_bcols(bu1), bu2nc=_bcols(-np.asarray(bu2, f)),
            br1c=_bcols(br1), br2c=_bcols(br2),
            bn1c=_bcols(bn1), bn2c=_bcols(bn2),
            bt1c=_bcols(bt1), bt2c=_bcols(bt2),
        )
    if not mask_ones:
        maskw = np.zeros((128, 128), f)
        maskw[DHALF:, :] = 1.0
        shared["maskw"] = maskw.astype(np.float16)

    QS = (nt + NQ - 1) // NQ
    bc = data.shape[0] // ncores
    # xs[p, t*BC+b] = data[b0+b, nt-1-t, p]
    xs_full = np.ascontiguousarray(data[:, ::-1, :].transpose(2, 1, 0))  # [IN,nt,B]
    in_maps = []
    for c in range(ncores):
        xs = np.zeros((128, NQ * QS, bc), np.float16)
        xs[:, :nt] = xs_full[:, :, c * bc : (c + 1) * bc].astype(np.float16)
        in_maps.append({**shared, "xs": xs})
    return in_maps, dts


def kernel(**inputs):
    """Full-input entry point: shards over 8 cores, runs the Bass kernel, gathers."""
    global _last_results
    inputs = {k: np.asarray(v) for k, v in inputs.items()}
    zero_bias = all(
        not np.any(np.asarray(inputs[k], np.float32))
        for k in ("bu1", "bu2", "br1", "br2", "bn1", "bn2", "bo1", "bo2",
                  "bt1", "bt2")
    )
    mask_ones = bool(
        np.all(np.asarray(inputs["data"], np.float32)[..., DHALF:].sum(-1) > 0)
    )
    in_maps, dts = make_inputs(**inputs, zero_bias=zero_bias, mask_ones=mask_ones)
    nc, _ = build_program(NT, dts, zero_bias=zero_bias, mask_ones=mask_ones)
    res = run_bass_kernel_spmd(nc, in_maps, core_ids=list(range(NCORES)))
    _last_results = res
    mean = np.concatenate([r["out_mean"] for r in res.results], axis=1)  # [LAT, B]
    std = np.concatenate([r["out_std"] for r in res.results], axis=1)
    return mean.T[None].astype(np.float32), std.T[None].astype(np.float32)
